# revision 16
# baseline (speedup 1.0000x reference)
"""BoundaryChunker Trainium2 kernel.

Strategy (data-parallel over batch: row r -> NeuronCore r):
  host:   per-row boundary positions, window starts s0 = max(0, t-4), combine
          coefficients c[u,g] = w[s0+g] / (wsum_u + eps) (zeroed outside the
          window / for padding slots), plus the cheap packed outputs
          (idx, conf, slot_mask).
  device: indirect-DMA gather of the 5-row contiguous window per boundary slot
          (split into taps 0-2 and 3-4 so the combine can start before the
          whole window landed), ACT+DVE weighted combine, PE transpose,
          fp32r matmul against resident W^T, DMA out.
  host:   slice to U slots, add bias, apply slot mask.

All compute ops are pinned to explicit engines: every engine's queue is
strict FIFO, so an op whose dependency resolves late (e.g. the last tile's
gather) must not sit ahead of earlier tiles' work in a shared queue.
"""

import os
import numpy as np

B, L, D = 8, 4096, 1024
POOL = 5
EPS = 1e-6
N_CORES = 8
P = 128
KC = D // P  # 8 contraction chunks
AUXC = 1 + POOL  # columns per m-tile in the aux tensor: offset + 5 coefs
TAP_SPLIT = 3  # gather taps [0:3) and [3:POOL) separately

# PE warm-up: junk bf16 matmuls keep HAM at K=8/8 while DMA/DVE phases run.
N_WARMUP_PRE = int(os.environ.get("KERNEL_WARMUP_PRE", "72"))
N_WARMUP_MID = int(os.environ.get("KERNEL_WARMUP_MID", "14"))

# Filled by the last kernel() call when BASS_KERNEL_PROFILE=1.
LAST_EXEC_NS = None
LAST_RESULTS = None

_nc_cache = {}


def _build_nc(m_tiles: int, n_valid: int):
    import concourse.bass as bass
    import concourse.mybir as mybir
    import concourse.tile as tile
    from concourse import bacc
    from concourse.masks import make_identity

    f32 = mybir.dt.float32
    f32r = mybir.dt.float32r
    bf16 = mybir.dt.bfloat16
    i32 = mybir.dt.int32
    Copy = mybir.ActivationFunctionType.Copy

    nc = bacc.Bacc("TRN2", target_bir_lowering=False, debug=False, num_devices=N_CORES)
    x_d = nc.dram_tensor("x", [L, D], f32, kind="ExternalInput").ap()
    # W^T pre-shuffled on host to [128, KC*D]: partition p holds rows
    # {p, 128+p, ...} of W^T back-to-back -> one DMA with 32KB-contiguous
    # per-partition reads (big packets win the SDMA round-robin vs gathers).
    wt_d = nc.dram_tensor("wt", [P, KC * D], f32r, kind="ExternalInput").ap()
    aux_d = nc.dram_tensor("aux", [P, AUXC * m_tiles], f32, kind="ExternalInput").ap()
    out_d = nc.dram_tensor("out", [n_valid, D], f32, kind="ExternalOutput").ap()

    def tile_rows(mt):
        return min(P, n_valid - mt * P)

    with tile.TileContext(nc) as tc:
        with (
            tc.tile_pool(name="const", bufs=1) as cpool,
            tc.tile_pool(name="w", bufs=1) as wpool,
            tc.tile_pool(name="gather", bufs=3) as gpool,
            tc.tile_pool(name="acc", bufs=2) as apool,
            tc.tile_pool(name="lhst", bufs=2) as lpool,
            tc.tile_pool(name="outsb", bufs=2) as opool,
            tc.tile_pool(name="pst", bufs=2, space="PSUM") as pst_pool,
            tc.tile_pool(name="pso", bufs=2, space="PSUM") as pso_pool,
            tc.tile_pool(name="pdum", bufs=1, space="PSUM") as pdum_pool,
        ):
            # W first: matmuls for every tile need it resident
            wt_sb = wpool.tile([P, KC * D], f32r, tag="w")
            nc.sync.dma_start(out=wt_sb[:], in_=wt_d)

            # aux via SWDGE so it starts during the sync-engine preamble
            aux_sb = cpool.tile([P, AUXC * m_tiles], f32, tag="aux")
            nc.gpsimd.dma_start(out=aux_sb[:], in_=aux_d)

            # gathers as early as possible (SWDGE queue, program order).
            # Full 128 descriptors per DMA even for padded slots: partial-
            # partition indirect DMA drains at a fraction of the rate.
            y_tiles = []
            for mt in range(m_tiles):
                y = gpool.tile([P, POOL * D], f32, tag="y")
                off_ap = aux_sb[:, mt * AUXC : mt * AUXC + 1].bitcast(i32)
                nc.gpsimd.indirect_dma_start(
                    out=y[:],
                    out_offset=None,
                    in_=x_d,
                    in_offset=bass.IndirectOffsetOnAxis(ap=off_ap, axis=0),
                )
                y_tiles.append(y)

            ident = cpool.tile([P, P], f32, tag="ident")
            make_identity(nc, ident[:])

            # PE warm-up spinner: junk bf16 matmuls with no data deps
            junk = cpool.tile([P, 512], bf16, tag="junk")
            nc.vector.memset(junk[:], 0.0)

            def spin(n):
                for _ in range(n):
                    pd = pdum_pool.tile([P, 512], f32, tag="pd")
                    nc.tensor.matmul(
                        pd[:], junk[:, :P], junk[:], start=True, stop=True
                    )

            spin(N_WARMUP_PRE)

            for mt in range(m_tiles):
                rows = tile_rows(mt)
                y = y_tiles[mt]
                c0 = mt * AUXC + 1
                acc = apool.tile([P, D], f32, tag="acc")
                # tap 0 on ACT (scaled copy), taps 1.. on DVE (fused FMA)
                nc.scalar.activation(
                    out=acc[:rows, :],
                    in_=y[:rows, 0:D],
                    func=Copy,
                    scale=aux_sb[:rows, c0 : c0 + 1],
                )
                for g in range(1, POOL):
                    nc.vector.affine_then_add(
                        out=acc[:rows, :],
                        in0=y[:rows, g * D : (g + 1) * D],
                        in1=acc[:rows, :],
                        scale=aux_sb[:rows, c0 + g : c0 + g + 1],
                        bias=0.0,
                    )

                # transposes into one wide PSUM tile, single ACT copy out
                pst = pst_pool.tile([P, D], f32, tag="pst")
                for k in range(KC):
                    nc.tensor.transpose(
                        out=pst[:, k * P : (k + 1) * P],
                        in_=acc[:, k * P : (k + 1) * P],
                        identity=ident[:],
                    )
                lt_all = lpool.tile([P, D], f32r, tag="lt")
                nc.scalar.activation(out=lt_all[:], in_=pst[:], func=Copy)

                out_sb = opool.tile([P, D], f32, tag="osb")
                for n in range(2):
                    pso = pso_pool.tile([P, 512], f32, tag="pso")
                    for k in range(KC):
                        nc.tensor.matmul(
                            pso[:],
                            lt_all[:, k * P : (k + 1) * P],
                            wt_sb[:, k * D + n * 512 : k * D + n * 512 + 512],
                            start=(k == 0),
                            stop=(k == KC - 1),
                        )
                    nc.scalar.activation(
                        out=out_sb[:rows, n * 512 : (n + 1) * 512],
                        in_=pso[:rows, :],
                        func=Copy,
                    )
                nc.sync.dma_start(
                    out=out_d[mt * P : mt * P + rows, :], in_=out_sb[:rows, :]
                )
                if mt + 1 < m_tiles:
                    spin(N_WARMUP_MID)
    nc.compile()
    return nc


def _host_prep(boundary_mask, change_score, boundary_confidence):
    """Per-row boundary packing metadata. All O(B*L) scalar work."""
    mask = np.asarray(boundary_mask).astype(bool)
    w = np.asarray(change_score).astype(np.float64)
    conf_in = np.asarray(boundary_confidence).astype(np.float32)

    counts = mask.sum(axis=1).astype(np.int64)
    U = max(int(counts.max()), 1)
    m_tiles = max(1, -(-U // P))

    aux = np.zeros((B, P, AUXC * m_tiles), dtype=np.float32)
    idx = np.zeros((B, U), dtype=np.int32)
    conf = np.zeros((B, U), dtype=np.float32)
    slot_mask = np.zeros((B, U), dtype=bool)

    g_off = np.arange(POOL, dtype=np.int64)
    for r in range(B):
        pos = np.nonzero(mask[r])[0]
        cnt = len(pos)
        if cnt == 0:
            continue
        idx[r, :cnt] = pos
        conf[r, :cnt] = conf_in[r, pos]
        slot_mask[r, :cnt] = True

        s0 = np.maximum(pos - (POOL - 1), 0)
        rows = s0[:, None] + g_off[None, :]  # (cnt, POOL)
        valid = rows <= pos[:, None]
        wrows = w[r, rows] * valid
        wsum = wrows.sum(axis=1)
        c = (wrows / (wsum + EPS)[:, None]).astype(np.float32)

        # slot u = mt*P + p  ->  aux[p, mt*AUXC] (offset), aux[p, mt*AUXC+1+g]
        u = np.arange(cnt)
        pp, mm = u % P, u // P
        aux[r, pp, mm * AUXC] = s0.astype(np.int32).view(np.float32)
        for g in range(POOL):
            aux[r, pp, mm * AUXC + 1 + g] = c[:, g]

    return counts, U, m_tiles, aux, idx, conf, slot_mask


def _install_ntff_hook_shim():
    """Provide antenv.axon_hooks (absent in this image) so bass_utils can
    NTFF-profile under axon. Mirrors trn_agent_boot's ctypes hook."""
    import sys

    if "antenv.axon_hooks" in sys.modules:
        return
    import contextlib
    import ctypes
    import types

    so_path = "/opt/axon/libaxon_pjrt.so"
    lib = ctypes.CDLL(so_path)
    if not hasattr(lib, "axon_start_nrt_profile"):
        raise RuntimeError("libaxon_pjrt.so lacks axon_start_nrt_profile")
    lib.axon_start_nrt_profile.argtypes = [
        ctypes.POINTER(ctypes.c_int64),
        ctypes.c_size_t,
    ]
    lib.axon_start_nrt_profile.restype = ctypes.c_int64
    lib.axon_stop_nrt_profile.argtypes = [ctypes.c_char_p]
    lib.axon_stop_nrt_profile.restype = ctypes.c_int64

    @contextlib.contextmanager
    def _hook(output_dir, device_ids):
        import jax

        jax.devices()
        if device_ids:
            ids = (ctypes.c_int64 * len(device_ids))(*device_ids)
            rc = lib.axon_start_nrt_profile(ids, len(device_ids))
        else:
            rc = lib.axon_start_nrt_profile(None, 0)
        if rc != 0:
            raise RuntimeError(f"axon_start_nrt_profile rc={rc}")
        try:
            yield
        finally:
            n = lib.axon_stop_nrt_profile(str(output_dir).encode())
            print(f"ntff profile: {n} file(s) written to {output_dir}")

    mod = types.ModuleType("antenv.axon_hooks")
    mod.get_axon_ntff_profile_hook = lambda: _hook
    mod.set_axon_ntff_profile_hook = lambda h: None
    sys.modules["antenv.axon_hooks"] = mod


def kernel(frame_embeddings, boundary_mask, change_score, boundary_confidence, W, b):
    global LAST_EXEC_NS, LAST_RESULTS
    from concourse.bass_utils import run_bass_kernel_spmd

    x = np.ascontiguousarray(np.asarray(frame_embeddings), dtype=np.float32)
    W = np.asarray(W).astype(np.float32)
    b = np.asarray(b).astype(np.float32)

    counts, U, m_tiles, aux, idx, conf, slot_mask = _host_prep(
        boundary_mask, change_score, boundary_confidence
    )

    key = (m_tiles, U)
    if key not in _nc_cache:
        _nc_cache[key] = _build_nc(m_tiles, U)
    nc = _nc_cache[key]

    # W^T shuffled: wt_shuf[p, k*D + n] = W.T[k*128 + p, n]
    wt = np.ascontiguousarray(
        W.T.reshape(KC, P, D).transpose(1, 0, 2).reshape(P, KC * D)
    )
    in_maps = [
        {"x": x[r], "wt": wt, "aux": aux[r]}
        for r in range(N_CORES)
    ]

    profile = os.environ.get("BASS_KERNEL_PROFILE", "0") == "1"
    if profile:
        try:
            _install_ntff_hook_shim()
        except Exception as e:
            print(f"ntff hook shim failed ({e}); running without profile")
            profile = False
    res = run_bass_kernel_spmd(
        nc, in_maps, list(range(N_CORES)), trace=profile
    )
    LAST_RESULTS = res
    LAST_EXEC_NS = res.exec_time_ns

    dev = np.stack([res.results[r]["out"] for r in range(N_CORES)], axis=0)
    chunks = dev + b[None, None, :]
    chunks = np.where(slot_mask[..., None], chunks, np.float32(0.0)).astype(np.float32)
    return chunks, slot_mask, idx, conf


# revision 19
# speedup vs baseline: 1.1248x; 1.1248x over previous
"""BoundaryChunker Trainium2 kernel.

Strategy (data-parallel over batch: row r -> NeuronCore r):
  host:   per-row boundary positions, window starts s0 = max(0, t-4), combine
          coefficients c[u,g] = w[s0+g] / (wsum_u + eps) (zeroed outside the
          window / for padding slots), plus the cheap packed outputs
          (idx, conf, slot_mask).
  device: indirect-DMA gather of the 5-row contiguous window per boundary slot
          (split into taps 0-2 and 3-4 so the combine can start before the
          whole window landed), ACT+DVE weighted combine, PE transpose,
          fp32r matmul against resident W^T, DMA out.
  host:   slice to U slots, add bias, apply slot mask.

All compute ops are pinned to explicit engines: every engine's queue is
strict FIFO, so an op whose dependency resolves late (e.g. the last tile's
gather) must not sit ahead of earlier tiles' work in a shared queue.
"""

import os
import numpy as np

B, L, D = 8, 4096, 1024
POOL = 5
EPS = 1e-6
N_CORES = 8
P = 128
KC = D // P  # 8 contraction chunks
AUXC = 1 + POOL  # columns per m-tile in the aux tensor: offset + 5 coefs
TAP_SPLIT = 3  # gather taps [0:3) and [3:POOL) separately

# PE warm-up: junk bf16 matmuls keep HAM at K=8/8 while DMA/DVE phases run.
N_WARMUP_PRE = int(os.environ.get("KERNEL_WARMUP_PRE", "130"))
N_WARMUP_MID = int(os.environ.get("KERNEL_WARMUP_MID", "0"))

# Filled by the last kernel() call when BASS_KERNEL_PROFILE=1.
LAST_EXEC_NS = None
LAST_RESULTS = None

_nc_cache = {}


def _build_nc(m_tiles: int, n_valid: int):
    import concourse.bass as bass
    import concourse.mybir as mybir
    import concourse.tile as tile
    from concourse import bacc
    from concourse.masks import make_identity

    f32 = mybir.dt.float32
    f32r = mybir.dt.float32r
    bf16 = mybir.dt.bfloat16
    i32 = mybir.dt.int32
    Copy = mybir.ActivationFunctionType.Copy

    nc = bacc.Bacc("TRN2", target_bir_lowering=False, debug=False, num_devices=N_CORES)
    x_d = nc.dram_tensor("x", [L, D], f32, kind="ExternalInput").ap()
    # W^T pre-shuffled on host to [128, KC*D]: partition p holds rows
    # {p, 128+p, ...} of W^T back-to-back -> one DMA with 32KB-contiguous
    # per-partition reads (big packets win the SDMA round-robin vs gathers).
    wt_d = nc.dram_tensor("wt", [P, KC * D], f32r, kind="ExternalInput").ap()
    aux_d = nc.dram_tensor("aux", [P, AUXC * m_tiles], f32, kind="ExternalInput").ap()
    out_d = nc.dram_tensor("out", [n_valid, D], f32, kind="ExternalOutput").ap()

    def tile_rows(mt):
        return min(P, n_valid - mt * P)

    with tile.TileContext(nc) as tc:
        with (
            tc.tile_pool(name="const", bufs=1) as cpool,
            tc.tile_pool(name="w", bufs=1) as wpool,
            tc.tile_pool(name="gather", bufs=3) as gpool,
            tc.tile_pool(name="acc", bufs=2) as apool,
            tc.tile_pool(name="lhst", bufs=2) as lpool,
            tc.tile_pool(name="outsb", bufs=2) as opool,
            tc.tile_pool(name="pst", bufs=2, space="PSUM") as pst_pool,
            tc.tile_pool(name="pso", bufs=2, space="PSUM") as pso_pool,
            tc.tile_pool(name="pdum", bufs=2, space="PSUM") as pdum_pool,
        ):
            # aux on the ACT HWDGE queue: separate FIFO from the W load, so
            # the tiny transfer completes immediately and unblocks gathers
            aux_sb = cpool.tile([P, AUXC * m_tiles], f32, tag="aux")
            nc.scalar.dma_start(out=aux_sb[:], in_=aux_d)

            # W on the sync HWDGE queue: matmuls for every tile need it
            wt_sb = wpool.tile([P, KC * D], f32r, tag="w")
            nc.sync.dma_start(out=wt_sb[:], in_=wt_d)

            # gathers as early as possible (SWDGE queue, program order).
            # Full 128 descriptors per DMA even for padded slots: partial-
            # partition indirect DMA drains at a fraction of the rate.
            y_tiles = []
            for mt in range(m_tiles):
                y = gpool.tile([P, POOL * D], f32, tag="y")
                off_ap = aux_sb[:, mt * AUXC : mt * AUXC + 1].bitcast(i32)
                nc.gpsimd.indirect_dma_start(
                    out=y[:],
                    out_offset=None,
                    in_=x_d,
                    in_offset=bass.IndirectOffsetOnAxis(ap=off_ap, axis=0),
                )
                y_tiles.append(y)

            ident = cpool.tile([P, P], f32, tag="ident")
            make_identity(nc, ident[:])

            # PE warm-up spinner: cheap junk bf16 matmuls with no data deps
            junk = cpool.tile([P, P], bf16, tag="junk")
            nc.vector.memset(junk[:], 0.0)

            def spin(n):
                for _ in range(n):
                    pd = pdum_pool.tile([P, P], f32, tag="pd")
                    nc.tensor.matmul(
                        pd[:], junk[:], junk[:], start=True, stop=True
                    )

            spin(N_WARMUP_PRE)

            for mt in range(m_tiles):
                rows = tile_rows(mt)
                y = y_tiles[mt]
                c0 = mt * AUXC + 1
                acc = apool.tile([P, D], f32, tag="acc")
                # tap 0 on ACT (scaled copy), taps 1.. on DVE (fused FMA)
                nc.scalar.activation(
                    out=acc[:rows, :],
                    in_=y[:rows, 0:D],
                    func=Copy,
                    scale=aux_sb[:rows, c0 : c0 + 1],
                )
                for g in range(1, POOL):
                    nc.vector.affine_then_add(
                        out=acc[:rows, :],
                        in0=y[:rows, g * D : (g + 1) * D],
                        in1=acc[:rows, :],
                        scale=aux_sb[:rows, c0 + g : c0 + g + 1],
                        bias=0.0,
                    )

                # transposes into one wide PSUM tile, single ACT copy out
                pst = pst_pool.tile([P, D], f32, tag="pst")
                for k in range(KC):
                    nc.tensor.transpose(
                        out=pst[:, k * P : (k + 1) * P],
                        in_=acc[:, k * P : (k + 1) * P],
                        identity=ident[:],
                    )
                lt_all = lpool.tile([P, D], f32r, tag="lt")
                nc.scalar.activation(out=lt_all[:], in_=pst[:], func=Copy)

                out_sb = opool.tile([P, D], f32, tag="osb")
                for n in range(2):
                    pso = pso_pool.tile([P, 512], f32, tag="pso")
                    for k in range(KC):
                        nc.tensor.matmul(
                            pso[:],
                            lt_all[:, k * P : (k + 1) * P],
                            wt_sb[:, k * D + n * 512 : k * D + n * 512 + 512],
                            start=(k == 0),
                            stop=(k == KC - 1),
                        )
                    nc.scalar.activation(
                        out=out_sb[:rows, n * 512 : (n + 1) * 512],
                        in_=pso[:rows, :],
                        func=Copy,
                    )
                nc.sync.dma_start(
                    out=out_d[mt * P : mt * P + rows, :], in_=out_sb[:rows, :]
                )
                if mt + 1 < m_tiles:
                    spin(N_WARMUP_MID)
    nc.compile()
    return nc


def _host_prep(boundary_mask, change_score, boundary_confidence):
    """Per-row boundary packing metadata. All O(B*L) scalar work."""
    mask = np.asarray(boundary_mask).astype(bool)
    w = np.asarray(change_score).astype(np.float64)
    conf_in = np.asarray(boundary_confidence).astype(np.float32)

    counts = mask.sum(axis=1).astype(np.int64)
    U = max(int(counts.max()), 1)
    m_tiles = max(1, -(-U // P))

    aux = np.zeros((B, P, AUXC * m_tiles), dtype=np.float32)
    idx = np.zeros((B, U), dtype=np.int32)
    conf = np.zeros((B, U), dtype=np.float32)
    slot_mask = np.zeros((B, U), dtype=bool)

    g_off = np.arange(POOL, dtype=np.int64)
    for r in range(B):
        pos = np.nonzero(mask[r])[0]
        cnt = len(pos)
        if cnt == 0:
            continue
        idx[r, :cnt] = pos
        conf[r, :cnt] = conf_in[r, pos]
        slot_mask[r, :cnt] = True

        s0 = np.maximum(pos - (POOL - 1), 0)
        rows = s0[:, None] + g_off[None, :]  # (cnt, POOL)
        valid = rows <= pos[:, None]
        wrows = w[r, rows] * valid
        wsum = wrows.sum(axis=1)
        c = (wrows / (wsum + EPS)[:, None]).astype(np.float32)

        # slot u = mt*P + p  ->  aux[p, mt*AUXC] (offset), aux[p, mt*AUXC+1+g]
        u = np.arange(cnt)
        pp, mm = u % P, u // P
        aux[r, pp, mm * AUXC] = s0.astype(np.int32).view(np.float32)
        for g in range(POOL):
            aux[r, pp, mm * AUXC + 1 + g] = c[:, g]

    return counts, U, m_tiles, aux, idx, conf, slot_mask


def _install_ntff_hook_shim():
    """Provide antenv.axon_hooks (absent in this image) so bass_utils can
    NTFF-profile under axon. Mirrors trn_agent_boot's ctypes hook."""
    import sys

    if "antenv.axon_hooks" in sys.modules:
        return
    import contextlib
    import ctypes
    import types

    so_path = "/opt/axon/libaxon_pjrt.so"
    lib = ctypes.CDLL(so_path)
    if not hasattr(lib, "axon_start_nrt_profile"):
        raise RuntimeError("libaxon_pjrt.so lacks axon_start_nrt_profile")
    lib.axon_start_nrt_profile.argtypes = [
        ctypes.POINTER(ctypes.c_int64),
        ctypes.c_size_t,
    ]
    lib.axon_start_nrt_profile.restype = ctypes.c_int64
    lib.axon_stop_nrt_profile.argtypes = [ctypes.c_char_p]
    lib.axon_stop_nrt_profile.restype = ctypes.c_int64

    @contextlib.contextmanager
    def _hook(output_dir, device_ids):
        import jax

        jax.devices()
        if device_ids:
            ids = (ctypes.c_int64 * len(device_ids))(*device_ids)
            rc = lib.axon_start_nrt_profile(ids, len(device_ids))
        else:
            rc = lib.axon_start_nrt_profile(None, 0)
        if rc != 0:
            raise RuntimeError(f"axon_start_nrt_profile rc={rc}")
        try:
            yield
        finally:
            n = lib.axon_stop_nrt_profile(str(output_dir).encode())
            print(f"ntff profile: {n} file(s) written to {output_dir}")

    mod = types.ModuleType("antenv.axon_hooks")
    mod.get_axon_ntff_profile_hook = lambda: _hook
    mod.set_axon_ntff_profile_hook = lambda h: None
    sys.modules["antenv.axon_hooks"] = mod


def kernel(frame_embeddings, boundary_mask, change_score, boundary_confidence, W, b):
    global LAST_EXEC_NS, LAST_RESULTS
    from concourse.bass_utils import run_bass_kernel_spmd

    x = np.ascontiguousarray(np.asarray(frame_embeddings), dtype=np.float32)
    W = np.asarray(W).astype(np.float32)
    b = np.asarray(b).astype(np.float32)

    counts, U, m_tiles, aux, idx, conf, slot_mask = _host_prep(
        boundary_mask, change_score, boundary_confidence
    )

    key = (m_tiles, U)
    if key not in _nc_cache:
        _nc_cache[key] = _build_nc(m_tiles, U)
    nc = _nc_cache[key]

    # W^T shuffled: wt_shuf[p, k*D + n] = W.T[k*128 + p, n]
    wt = np.ascontiguousarray(
        W.T.reshape(KC, P, D).transpose(1, 0, 2).reshape(P, KC * D)
    )
    in_maps = [
        {"x": x[r], "wt": wt, "aux": aux[r]}
        for r in range(N_CORES)
    ]

    profile = os.environ.get("BASS_KERNEL_PROFILE", "0") == "1"
    if profile:
        try:
            _install_ntff_hook_shim()
        except Exception as e:
            print(f"ntff hook shim failed ({e}); running without profile")
            profile = False
    res = run_bass_kernel_spmd(
        nc, in_maps, list(range(N_CORES)), trace=profile
    )
    LAST_RESULTS = res
    LAST_EXEC_NS = res.exec_time_ns

    dev = np.stack([res.results[r]["out"] for r in range(N_CORES)], axis=0)
    chunks = dev + b[None, None, :]
    chunks = np.where(slot_mask[..., None], chunks, np.float32(0.0)).astype(np.float32)
    return chunks, slot_mask, idx, conf


# revision 20
# speedup vs baseline: 1.2360x; 1.0989x over previous
"""BoundaryChunker Trainium2 kernel.

Strategy (data-parallel over batch: row r -> NeuronCore r):
  host:   per-row boundary positions, window starts s0 = max(0, t-4), combine
          coefficients c[u,g] = w[s0+g] / (wsum_u + eps) (zeroed outside the
          window / for padding slots), plus the cheap packed outputs
          (idx, conf, slot_mask).
  device: indirect-DMA gather of the 5-row contiguous window per boundary slot
          (split into taps 0-2 and 3-4 so the combine can start before the
          whole window landed), ACT+DVE weighted combine, PE transpose,
          fp32r matmul against resident W^T, DMA out.
  host:   slice to U slots, add bias, apply slot mask.

All compute ops are pinned to explicit engines: every engine's queue is
strict FIFO, so an op whose dependency resolves late (e.g. the last tile's
gather) must not sit ahead of earlier tiles' work in a shared queue.
"""

import os
import numpy as np

B, L, D = 8, 4096, 1024
POOL = 5
EPS = 1e-6
N_CORES = 8
P = 128
KC = D // P  # 8 contraction chunks
AUXC = 1 + POOL  # columns per m-tile in the aux tensor: offset + 5 coefs
TAP_SPLIT = 3  # gather taps [0:3) and [3:POOL) separately

# PE warm-up: junk bf16 matmuls keep HAM at K=8/8 while DMA/DVE phases run.
N_WARMUP_PRE = int(os.environ.get("KERNEL_WARMUP_PRE", "130"))
N_WARMUP_MID = int(os.environ.get("KERNEL_WARMUP_MID", "0"))

# Filled by the last kernel() call when BASS_KERNEL_PROFILE=1.
LAST_EXEC_NS = None
LAST_RESULTS = None

_nc_cache = {}


def _build_nc(m_tiles: int, n_valid: int):
    import concourse.bass as bass
    import concourse.mybir as mybir
    import concourse.tile as tile
    from concourse import bacc
    from concourse.masks import make_identity

    f32 = mybir.dt.float32
    f32r = mybir.dt.float32r
    bf16 = mybir.dt.bfloat16
    i32 = mybir.dt.int32
    Copy = mybir.ActivationFunctionType.Copy

    nc = bacc.Bacc("TRN2", target_bir_lowering=False, debug=False, num_devices=N_CORES)
    x_d = nc.dram_tensor("x", [L, D], f32, kind="ExternalInput").ap()
    # W^T pre-shuffled on host to [128, KC*D]: partition p holds rows
    # {p, 128+p, ...} of W^T back-to-back -> one DMA with 32KB-contiguous
    # per-partition reads (big packets win the SDMA round-robin vs gathers).
    wt_d = nc.dram_tensor("wt", [P, KC * D], f32r, kind="ExternalInput").ap()
    aux_d = nc.dram_tensor("aux", [P, AUXC * m_tiles], f32, kind="ExternalInput").ap()
    out_d = nc.dram_tensor("out", [n_valid, D], f32, kind="ExternalOutput").ap()

    def tile_rows(mt):
        return min(P, n_valid - mt * P)

    with tile.TileContext(nc) as tc:
        with (
            tc.tile_pool(name="const", bufs=1) as cpool,
            tc.tile_pool(name="w", bufs=1) as wpool,
            tc.tile_pool(name="gather", bufs=3) as gpool,
            tc.tile_pool(name="acc", bufs=2) as apool,
            tc.tile_pool(name="lhst", bufs=2) as lpool,
            tc.tile_pool(name="outsb", bufs=2) as opool,
            tc.tile_pool(name="pst", bufs=2, space="PSUM") as pst_pool,
            tc.tile_pool(name="pso", bufs=2, space="PSUM") as pso_pool,
            tc.tile_pool(name="pdum", bufs=2, space="PSUM") as pdum_pool,
        ):
            # aux alone on the ACT HWDGE queue: the tiny transfer completes
            # immediately (nothing else floods HBM yet) and unblocks gathers
            aux_sb = cpool.tile([P, AUXC * m_tiles], f32, tag="aux")
            nc.scalar.dma_start(out=aux_sb[:], in_=aux_d)

            # identity before any Q7 DMA issues so transposes aren't gated
            # behind the gather/W descriptor generation
            ident = cpool.tile([P, P], f32, tag="ident")
            make_identity(nc, ident[:])

            # All heavy HBM traffic on the ONE SWDGE queue, in priority
            # order: the per-engine descriptor rings drain strictly in issue
            # order, so gathers get full bandwidth first and W streams in
            # right behind them, chunk by chunk, just in time for the
            # matmuls. (Putting W on a parallel HWDGE queue instead makes
            # the two streams round-robin and everything arrives late.)
            y_tiles = []
            for mt in range(m_tiles):
                y = gpool.tile([P, POOL * D], f32, tag="y")
                off_ap = aux_sb[:, mt * AUXC : mt * AUXC + 1].bitcast(i32)
                nc.gpsimd.indirect_dma_start(
                    out=y[:],
                    out_offset=None,
                    in_=x_d,
                    in_offset=bass.IndirectOffsetOnAxis(ap=off_ap, axis=0),
                )
                y_tiles.append(y)

            wt_sb = wpool.tile([P, KC * D], f32r, tag="w")
            for k in range(KC):
                nc.gpsimd.dma_start(
                    out=wt_sb[:, k * D : (k + 1) * D],
                    in_=wt_d[:, k * D : (k + 1) * D],
                )

            # PE warm-up spinner: cheap junk bf16 matmuls with no data deps
            junk = cpool.tile([P, P], bf16, tag="junk")
            nc.vector.memset(junk[:], 0.0)

            def spin(n):
                for _ in range(n):
                    pd = pdum_pool.tile([P, P], f32, tag="pd")
                    nc.tensor.matmul(
                        pd[:], junk[:], junk[:], start=True, stop=True
                    )

            spin(N_WARMUP_PRE)

            for mt in range(m_tiles):
                rows = tile_rows(mt)
                y = y_tiles[mt]
                c0 = mt * AUXC + 1
                acc = apool.tile([P, D], f32, tag="acc")
                # tap 0 on ACT (scaled copy), taps 1.. on DVE (fused FMA)
                nc.scalar.activation(
                    out=acc[:rows, :],
                    in_=y[:rows, 0:D],
                    func=Copy,
                    scale=aux_sb[:rows, c0 : c0 + 1],
                )
                for g in range(1, POOL):
                    nc.vector.affine_then_add(
                        out=acc[:rows, :],
                        in0=y[:rows, g * D : (g + 1) * D],
                        in1=acc[:rows, :],
                        scale=aux_sb[:rows, c0 + g : c0 + g + 1],
                        bias=0.0,
                    )

                # transposes into one wide PSUM tile, single ACT copy out
                pst = pst_pool.tile([P, D], f32, tag="pst")
                for k in range(KC):
                    nc.tensor.transpose(
                        out=pst[:, k * P : (k + 1) * P],
                        in_=acc[:, k * P : (k + 1) * P],
                        identity=ident[:],
                    )
                lt_all = lpool.tile([P, D], f32r, tag="lt")
                nc.scalar.activation(out=lt_all[:], in_=pst[:], func=Copy)

                out_sb = opool.tile([P, D], f32, tag="osb")
                for n in range(2):
                    pso = pso_pool.tile([P, 512], f32, tag="pso")
                    for k in range(KC):
                        nc.tensor.matmul(
                            pso[:],
                            lt_all[:, k * P : (k + 1) * P],
                            wt_sb[:, k * D + n * 512 : k * D + n * 512 + 512],
                            start=(k == 0),
                            stop=(k == KC - 1),
                        )
                    nc.scalar.activation(
                        out=out_sb[:rows, n * 512 : (n + 1) * 512],
                        in_=pso[:rows, :],
                        func=Copy,
                    )
                nc.sync.dma_start(
                    out=out_d[mt * P : mt * P + rows, :], in_=out_sb[:rows, :]
                )
                if mt + 1 < m_tiles:
                    spin(N_WARMUP_MID)
    nc.compile()
    return nc


def _host_prep(boundary_mask, change_score, boundary_confidence):
    """Per-row boundary packing metadata. All O(B*L) scalar work."""
    mask = np.asarray(boundary_mask).astype(bool)
    w = np.asarray(change_score).astype(np.float64)
    conf_in = np.asarray(boundary_confidence).astype(np.float32)

    counts = mask.sum(axis=1).astype(np.int64)
    U = max(int(counts.max()), 1)
    m_tiles = max(1, -(-U // P))

    aux = np.zeros((B, P, AUXC * m_tiles), dtype=np.float32)
    idx = np.zeros((B, U), dtype=np.int32)
    conf = np.zeros((B, U), dtype=np.float32)
    slot_mask = np.zeros((B, U), dtype=bool)

    g_off = np.arange(POOL, dtype=np.int64)
    for r in range(B):
        pos = np.nonzero(mask[r])[0]
        cnt = len(pos)
        if cnt == 0:
            continue
        idx[r, :cnt] = pos
        conf[r, :cnt] = conf_in[r, pos]
        slot_mask[r, :cnt] = True

        s0 = np.maximum(pos - (POOL - 1), 0)
        rows = s0[:, None] + g_off[None, :]  # (cnt, POOL)
        valid = rows <= pos[:, None]
        wrows = w[r, rows] * valid
        wsum = wrows.sum(axis=1)
        c = (wrows / (wsum + EPS)[:, None]).astype(np.float32)

        # slot u = mt*P + p  ->  aux[p, mt*AUXC] (offset), aux[p, mt*AUXC+1+g]
        u = np.arange(cnt)
        pp, mm = u % P, u // P
        aux[r, pp, mm * AUXC] = s0.astype(np.int32).view(np.float32)
        for g in range(POOL):
            aux[r, pp, mm * AUXC + 1 + g] = c[:, g]

    return counts, U, m_tiles, aux, idx, conf, slot_mask


def _install_ntff_hook_shim():
    """Provide antenv.axon_hooks (absent in this image) so bass_utils can
    NTFF-profile under axon. Mirrors trn_agent_boot's ctypes hook."""
    import sys

    if "antenv.axon_hooks" in sys.modules:
        return
    import contextlib
    import ctypes
    import types

    so_path = "/opt/axon/libaxon_pjrt.so"
    lib = ctypes.CDLL(so_path)
    if not hasattr(lib, "axon_start_nrt_profile"):
        raise RuntimeError("libaxon_pjrt.so lacks axon_start_nrt_profile")
    lib.axon_start_nrt_profile.argtypes = [
        ctypes.POINTER(ctypes.c_int64),
        ctypes.c_size_t,
    ]
    lib.axon_start_nrt_profile.restype = ctypes.c_int64
    lib.axon_stop_nrt_profile.argtypes = [ctypes.c_char_p]
    lib.axon_stop_nrt_profile.restype = ctypes.c_int64

    @contextlib.contextmanager
    def _hook(output_dir, device_ids):
        import jax

        jax.devices()
        if device_ids:
            ids = (ctypes.c_int64 * len(device_ids))(*device_ids)
            rc = lib.axon_start_nrt_profile(ids, len(device_ids))
        else:
            rc = lib.axon_start_nrt_profile(None, 0)
        if rc != 0:
            raise RuntimeError(f"axon_start_nrt_profile rc={rc}")
        try:
            yield
        finally:
            n = lib.axon_stop_nrt_profile(str(output_dir).encode())
            print(f"ntff profile: {n} file(s) written to {output_dir}")

    mod = types.ModuleType("antenv.axon_hooks")
    mod.get_axon_ntff_profile_hook = lambda: _hook
    mod.set_axon_ntff_profile_hook = lambda h: None
    sys.modules["antenv.axon_hooks"] = mod


def kernel(frame_embeddings, boundary_mask, change_score, boundary_confidence, W, b):
    global LAST_EXEC_NS, LAST_RESULTS
    from concourse.bass_utils import run_bass_kernel_spmd

    x = np.ascontiguousarray(np.asarray(frame_embeddings), dtype=np.float32)
    W = np.asarray(W).astype(np.float32)
    b = np.asarray(b).astype(np.float32)

    counts, U, m_tiles, aux, idx, conf, slot_mask = _host_prep(
        boundary_mask, change_score, boundary_confidence
    )

    key = (m_tiles, U)
    if key not in _nc_cache:
        _nc_cache[key] = _build_nc(m_tiles, U)
    nc = _nc_cache[key]

    # W^T shuffled: wt_shuf[p, k*D + n] = W.T[k*128 + p, n]
    wt = np.ascontiguousarray(
        W.T.reshape(KC, P, D).transpose(1, 0, 2).reshape(P, KC * D)
    )
    in_maps = [
        {"x": x[r], "wt": wt, "aux": aux[r]}
        for r in range(N_CORES)
    ]

    profile = os.environ.get("BASS_KERNEL_PROFILE", "0") == "1"
    if profile:
        try:
            _install_ntff_hook_shim()
        except Exception as e:
            print(f"ntff hook shim failed ({e}); running without profile")
            profile = False
    res = run_bass_kernel_spmd(
        nc, in_maps, list(range(N_CORES)), trace=profile
    )
    LAST_RESULTS = res
    LAST_EXEC_NS = res.exec_time_ns

    dev = np.stack([res.results[r]["out"] for r in range(N_CORES)], axis=0)
    chunks = dev + b[None, None, :]
    chunks = np.where(slot_mask[..., None], chunks, np.float32(0.0)).astype(np.float32)
    return chunks, slot_mask, idx, conf


# revision 21
# speedup vs baseline: 1.5045x; 1.2172x over previous
"""BoundaryChunker Trainium2 kernel.

Strategy (data-parallel over batch: row r -> NeuronCore r):
  host:   per-row boundary positions, window starts s0 = max(0, t-4), combine
          coefficients c[u,g] = w[s0+g] / (wsum_u + eps) (zeroed outside the
          window / for padding slots), plus the cheap packed outputs
          (idx, conf, slot_mask). x and W^T are cast to fp16 on host.
  device: indirect-DMA gather of the 5-row contiguous window per boundary slot
          (one 10KB descriptor per slot), ACT+DVE weighted combine in fp32,
          PE transpose, fp16 matmul against resident W^T, DMA out.
  host:   slice to U slots, add bias, apply slot mask.

All compute ops are pinned to explicit engines: every engine's queue is
strict FIFO, so an op whose dependency resolves late (e.g. the last tile's
gather) must not sit ahead of earlier tiles' work in a shared queue.

All heavy HBM traffic rides the single SWDGE queue in priority order
(gathers, then W chunk by chunk): its per-engine descriptor rings drain
strictly in issue order, which both keeps the tiny aux load (ACT HWDGE
queue) from being starved and lets matmuls pace behind per-chunk W arrival.
"""

import os
import numpy as np

B, L, D = 8, 4096, 1024
POOL = 5
EPS = 1e-6
N_CORES = 8
P = 128
KC = D // P  # 8 contraction chunks
AUXC = 1 + POOL  # columns per m-tile in the aux tensor: offset + 5 coefs
TAP_SPLIT = 3  # last gather split into taps [0:3) / [3:POOL)

# "fp16" (fast, ~4e-4 rel err) or "fp32r" (~1.4e-4 rel err, ~2x matmul time)
MM_DTYPE = os.environ.get("KERNEL_MM_DTYPE", "fp16")

# PE warm-up: junk bf16 matmuls keep HAM at K=8/8 while DMA/DVE phases run.
N_WARMUP_PRE = int(os.environ.get("KERNEL_WARMUP_PRE", "90"))

# Filled by the last kernel() call when BASS_KERNEL_PROFILE=1.
LAST_EXEC_NS = None
LAST_RESULTS = None

_nc_cache = {}


def _build_nc(m_tiles: int, n_valid: int):
    import concourse.bass as bass
    import concourse.mybir as mybir
    import concourse.tile as tile
    from concourse import bacc
    from concourse.masks import make_identity

    f32 = mybir.dt.float32
    bf16 = mybir.dt.bfloat16
    i32 = mybir.dt.int32
    Copy = mybir.ActivationFunctionType.Copy
    if MM_DTYPE == "fp16":
        xdt = mmdt = mybir.dt.float16
    else:
        xdt, mmdt = f32, mybir.dt.float32r

    nc = bacc.Bacc("TRN2", target_bir_lowering=False, debug=False, num_devices=N_CORES)
    x_d = nc.dram_tensor("x", [L, D], xdt, kind="ExternalInput").ap()
    # W^T pre-shuffled on host to [128, KC*D]: partition p holds rows
    # {p, 128+p, ...} of W^T back-to-back (contiguous per-partition reads).
    wt_d = nc.dram_tensor("wt", [P, KC * D], mmdt, kind="ExternalInput").ap()
    aux_d = nc.dram_tensor("aux", [P, AUXC * m_tiles], f32, kind="ExternalInput").ap()
    out_d = nc.dram_tensor("out", [n_valid, D], f32, kind="ExternalOutput").ap()

    def tile_rows(mt):
        return min(P, n_valid - mt * P)

    with tile.TileContext(nc) as tc:
        with (
            tc.tile_pool(name="const", bufs=1) as cpool,
            tc.tile_pool(name="w", bufs=1) as wpool,
            tc.tile_pool(name="gather", bufs=3) as gpool,
            tc.tile_pool(name="acc", bufs=2) as apool,
            tc.tile_pool(name="lhst", bufs=2) as lpool,
            tc.tile_pool(name="outsb", bufs=2) as opool,
            tc.tile_pool(name="pst", bufs=2, space="PSUM") as pst_pool,
            tc.tile_pool(name="pso", bufs=2, space="PSUM") as pso_pool,
            tc.tile_pool(name="pdum", bufs=2, space="PSUM") as pdum_pool,
        ):
            # aux alone on the ACT HWDGE queue: the tiny transfer completes
            # immediately (nothing else floods HBM yet) and unblocks gathers
            aux_sb = cpool.tile([P, AUXC * m_tiles], f32, tag="aux")
            nc.scalar.dma_start(out=aux_sb[:], in_=aux_d)

            # identity before any Q7 DMA issues so transposes aren't gated
            # behind the gather/W descriptor generation
            ident = cpool.tile([P, P], f32, tag="ident")
            make_identity(nc, ident[:])

            # SWDGE queue, in priority order: gathers (last one split by taps
            # so its combine starts before the final bytes land), then W.
            y_tiles = []
            for mt in range(m_tiles):
                y = gpool.tile([P, POOL * D], xdt, tag="y")
                off_ap = aux_sb[:, mt * AUXC : mt * AUXC + 1].bitcast(i32)
                if mt == m_tiles - 1:
                    nc.gpsimd.indirect_dma_start(
                        out=y[:, : TAP_SPLIT * D],
                        out_offset=None,
                        in_=x_d,
                        in_offset=bass.IndirectOffsetOnAxis(ap=off_ap, axis=0),
                    )
                    nc.gpsimd.indirect_dma_start(
                        out=y[:, TAP_SPLIT * D :],
                        out_offset=None,
                        in_=x_d,
                        in_offset=bass.IndirectOffsetOnAxis(ap=off_ap, axis=0),
                        element_offset=TAP_SPLIT * D,
                    )
                else:
                    nc.gpsimd.indirect_dma_start(
                        out=y[:],
                        out_offset=None,
                        in_=x_d,
                        in_offset=bass.IndirectOffsetOnAxis(ap=off_ap, axis=0),
                    )
                y_tiles.append(y)

            wt_sb = wpool.tile([P, KC * D], mmdt, tag="w")
            for k in range(KC):
                nc.gpsimd.dma_start(
                    out=wt_sb[:, k * D : (k + 1) * D],
                    in_=wt_d[:, k * D : (k + 1) * D],
                )

            # PE warm-up spinner: cheap junk bf16 matmuls with no data deps
            junk = cpool.tile([P, P], bf16, tag="junk")
            nc.vector.memset(junk[:], 0.0)
            for _ in range(N_WARMUP_PRE):
                pd = pdum_pool.tile([P, P], f32, tag="pd")
                nc.tensor.matmul(pd[:], junk[:], junk[:], start=True, stop=True)

            for mt in range(m_tiles):
                rows = tile_rows(mt)
                y = y_tiles[mt]
                c0 = mt * AUXC + 1
                acc = apool.tile([P, D], f32, tag="acc")
                # tap 0 on ACT (scaled copy), taps 1.. on DVE (fused FMA)
                nc.scalar.activation(
                    out=acc[:rows, :],
                    in_=y[:rows, 0:D],
                    func=Copy,
                    scale=aux_sb[:rows, c0 : c0 + 1],
                )
                for g in range(1, POOL):
                    nc.vector.affine_then_add(
                        out=acc[:rows, :],
                        in0=y[:rows, g * D : (g + 1) * D],
                        in1=acc[:rows, :],
                        scale=aux_sb[:rows, c0 + g : c0 + g + 1],
                        bias=0.0,
                    )

                # transposes into one wide PSUM tile; copy out in two halves
                # so the first matmuls start while later transposes still run
                pst = pst_pool.tile([P, D], f32, tag="pst")
                lt_all = lpool.tile([P, D], mmdt, tag="lt")
                for k in range(KC):
                    nc.tensor.transpose(
                        out=pst[:, k * P : (k + 1) * P],
                        in_=acc[:, k * P : (k + 1) * P],
                        identity=ident[:],
                    )
                    if k == KC // 2 - 1:
                        nc.scalar.activation(
                            out=lt_all[:, : D // 2], in_=pst[:, : D // 2], func=Copy
                        )
                nc.scalar.activation(
                    out=lt_all[:, D // 2 :], in_=pst[:, D // 2 :], func=Copy
                )

                out_sb = opool.tile([P, D], f32, tag="osb")
                for n in range(2):
                    pso = pso_pool.tile([P, 512], f32, tag="pso")
                    for k in range(KC):
                        nc.tensor.matmul(
                            pso[:],
                            lt_all[:, k * P : (k + 1) * P],
                            wt_sb[:, k * D + n * 512 : k * D + n * 512 + 512],
                            start=(k == 0),
                            stop=(k == KC - 1),
                        )
                    nc.scalar.activation(
                        out=out_sb[:rows, n * 512 : (n + 1) * 512],
                        in_=pso[:rows, :],
                        func=Copy,
                    )
                nc.sync.dma_start(
                    out=out_d[mt * P : mt * P + rows, :], in_=out_sb[:rows, :]
                )
    nc.compile()
    return nc


def _host_prep(boundary_mask, change_score, boundary_confidence):
    """Per-row boundary packing metadata. All O(B*L) scalar work."""
    mask = np.asarray(boundary_mask).astype(bool)
    w = np.asarray(change_score).astype(np.float64)
    conf_in = np.asarray(boundary_confidence).astype(np.float32)

    counts = mask.sum(axis=1).astype(np.int64)
    U = max(int(counts.max()), 1)
    m_tiles = max(1, -(-U // P))

    aux = np.zeros((B, P, AUXC * m_tiles), dtype=np.float32)
    idx = np.zeros((B, U), dtype=np.int32)
    conf = np.zeros((B, U), dtype=np.float32)
    slot_mask = np.zeros((B, U), dtype=bool)

    g_off = np.arange(POOL, dtype=np.int64)
    for r in range(B):
        pos = np.nonzero(mask[r])[0]
        cnt = len(pos)
        if cnt == 0:
            continue
        idx[r, :cnt] = pos
        conf[r, :cnt] = conf_in[r, pos]
        slot_mask[r, :cnt] = True

        s0 = np.maximum(pos - (POOL - 1), 0)
        rows = s0[:, None] + g_off[None, :]  # (cnt, POOL)
        valid = rows <= pos[:, None]
        wrows = w[r, rows] * valid
        wsum = wrows.sum(axis=1)
        c = (wrows / (wsum + EPS)[:, None]).astype(np.float32)

        # slot u = mt*P + p  ->  aux[p, mt*AUXC] (offset), aux[p, mt*AUXC+1+g]
        u = np.arange(cnt)
        pp, mm = u % P, u // P
        aux[r, pp, mm * AUXC] = s0.astype(np.int32).view(np.float32)
        for g in range(POOL):
            aux[r, pp, mm * AUXC + 1 + g] = c[:, g]

    return counts, U, m_tiles, aux, idx, conf, slot_mask


def _install_ntff_hook_shim():
    """Provide antenv.axon_hooks (absent in this image) so bass_utils can
    NTFF-profile under axon. Mirrors trn_agent_boot's ctypes hook."""
    import sys

    if "antenv.axon_hooks" in sys.modules:
        return
    import contextlib
    import ctypes
    import types

    so_path = "/opt/axon/libaxon_pjrt.so"
    lib = ctypes.CDLL(so_path)
    if not hasattr(lib, "axon_start_nrt_profile"):
        raise RuntimeError("libaxon_pjrt.so lacks axon_start_nrt_profile")
    lib.axon_start_nrt_profile.argtypes = [
        ctypes.POINTER(ctypes.c_int64),
        ctypes.c_size_t,
    ]
    lib.axon_start_nrt_profile.restype = ctypes.c_int64
    lib.axon_stop_nrt_profile.argtypes = [ctypes.c_char_p]
    lib.axon_stop_nrt_profile.restype = ctypes.c_int64

    @contextlib.contextmanager
    def _hook(output_dir, device_ids):
        import jax

        jax.devices()
        if device_ids:
            ids = (ctypes.c_int64 * len(device_ids))(*device_ids)
            rc = lib.axon_start_nrt_profile(ids, len(device_ids))
        else:
            rc = lib.axon_start_nrt_profile(None, 0)
        if rc != 0:
            raise RuntimeError(f"axon_start_nrt_profile rc={rc}")
        try:
            yield
        finally:
            n = lib.axon_stop_nrt_profile(str(output_dir).encode())
            print(f"ntff profile: {n} file(s) written to {output_dir}")

    mod = types.ModuleType("antenv.axon_hooks")
    mod.get_axon_ntff_profile_hook = lambda: _hook
    mod.set_axon_ntff_profile_hook = lambda h: None
    sys.modules["antenv.axon_hooks"] = mod


def kernel(frame_embeddings, boundary_mask, change_score, boundary_confidence, W, b):
    global LAST_EXEC_NS, LAST_RESULTS
    from concourse.bass_utils import run_bass_kernel_spmd

    W = np.asarray(W).astype(np.float32)
    b = np.asarray(b).astype(np.float32)
    xdt = np.float16 if MM_DTYPE == "fp16" else np.float32
    x = np.ascontiguousarray(np.asarray(frame_embeddings), dtype=xdt)

    counts, U, m_tiles, aux, idx, conf, slot_mask = _host_prep(
        boundary_mask, change_score, boundary_confidence
    )

    key = (m_tiles, U, MM_DTYPE)
    if key not in _nc_cache:
        _nc_cache[key] = _build_nc(m_tiles, U)
    nc = _nc_cache[key]

    # W^T shuffled: wt_shuf[p, k*D + n] = W.T[k*128 + p, n]
    wt = np.ascontiguousarray(
        W.T.reshape(KC, P, D).transpose(1, 0, 2).reshape(P, KC * D).astype(xdt)
    )
    in_maps = [
        {"x": x[r], "wt": wt, "aux": aux[r]}
        for r in range(N_CORES)
    ]

    profile = os.environ.get("BASS_KERNEL_PROFILE", "0") == "1"
    if profile:
        try:
            _install_ntff_hook_shim()
        except Exception as e:
            print(f"ntff hook shim failed ({e}); running without profile")
            profile = False
    res = run_bass_kernel_spmd(
        nc, in_maps, list(range(N_CORES)), trace=profile
    )
    LAST_RESULTS = res
    LAST_EXEC_NS = res.exec_time_ns

    dev = np.stack([res.results[r]["out"] for r in range(N_CORES)], axis=0)
    chunks = dev + b[None, None, :]
    chunks = np.where(slot_mask[..., None], chunks, np.float32(0.0)).astype(np.float32)
    return chunks, slot_mask, idx, conf
